# revision 15
# baseline (speedup 1.0000x reference)
"""Multi-head attention (B=2, S=2048, D=1024, H=16, causal + rel-pos-bias + RoPE)
on 8 Trainium2 NeuronCores.

Sharding: core c handles batch c//4 and head-group c%4 (4 heads = 256 model dims).
Each core computes its heads' Q/K/V projections (column-sharded weights), RoPE,
causal attention with relative position bias, and a partial output projection
(row-sharded Wo). Host sums the 4 partials per batch and adds Wo_b.
"""

import math

import numpy as np
import ml_dtypes

import concourse.bass as bass
import concourse.mybir as mybir
import concourse.tile as tile
from concourse import bacc
from concourse.bass_utils import run_bass_kernel_spmd

BF16 = ml_dtypes.bfloat16
FP8E3 = ml_dtypes.float8_e3m4

B, S, D, H = 2, 2048, 1024, 16
DK = 64
SCALE = math.sqrt(DK)
HPC = 4          # heads per core
GDIM = HPC * DK  # 256 model dims per core
N_CORES = 8
KT = S // 128    # 16 k-tiles
QC = S // 512    # 4 q-chunks

f32 = mybir.dt.float32
f32r = mybir.dt.float32r
bf16 = mybir.dt.bfloat16


def _sched():
    """Attention tile schedule, shared by host bias packer and device builder.

    Yields (h, qc, kt, n, q0): head-local index, q-chunk, k-tile, the valid
    column count and starting q of the S^T tile [128 k, n q]."""
    for h in range(HPC):
        for qc in range(QC):
            for kt in range(4 * qc + 4):
                if kt // 4 == qc:  # diagonal-crossing tile
                    n = 512 - 128 * (kt % 4)
                    q0 = 128 * kt
                else:
                    n = 512
                    q0 = 512 * qc
                yield h, qc, kt, n, q0


EB_PER_HEAD = sum(128 * n for h, qc, kt, n, q0 in _sched()) // HPC
EB_TOTAL = EB_PER_HEAD * HPC

_PROGRAM = None


def _quads(qc):
    """kt quad-groups for one (h, qc) chunk: list of [(kt,n,q0)...]."""
    kts = list(range(4 * qc + 4))
    out = []
    for i in range(0, len(kts), 4):
        grp = []
        for kt in kts[i:i + 4]:
            if kt // 4 == qc:
                n = 512 - 128 * (kt % 4)
                q0 = 128 * kt
            else:
                n = 512
                q0 = 512 * qc
            grp.append((kt, n, q0))
        out.append(grp)
    return out


def _build_program():
    nc = bacc.Bacc("TRN2", target_bir_lowering=False, debug=False)

    dqT = nc.dram_tensor("qT", [128, 2, 8, 1024], bf16,
                         kind="ExternalInput").ap()
    dkT = nc.dram_tensor("kT", [128, 2, 8, 1024], bf16,
                         kind="ExternalInput").ap()
    dvT = nc.dram_tensor("vT", [4, 128, 8, 512], bf16,
                         kind="ExternalInput").ap()
    dwq = nc.dram_tensor("wq", [128, 8, GDIM], bf16,
                         kind="ExternalInput").ap()
    dwk = nc.dram_tensor("wk", [128, 8, GDIM], bf16,
                         kind="ExternalInput").ap()
    dwv = nc.dram_tensor("wv", [128, 8, GDIM], bf16,
                         kind="ExternalInput").ap()
    dwo = nc.dram_tensor("wo", [128, 2, D], bf16, kind="ExternalInput").ap()
    deb = nc.dram_tensor("eb", [EB_TOTAL], mybir.dt.float8e3,
                         kind="ExternalInput").ap()
    dcos = nc.dram_tensor("cosT", [128, S], bf16, kind="ExternalInput").ap()
    dsin = nc.dram_tensor("sinT", [128, S], bf16, kind="ExternalInput").ap()
    dout = nc.dram_tensor("out", [S, D], f32, kind="ExternalOutput").ap()

    with tile.TileContext(nc) as tc:
        with tc.tile_pool(name="consts", bufs=1) as consts, \
             tc.tile_pool(name="persist", bufs=1) as persist, \
             tc.tile_pool(name="ropep", bufs=2) as ropep, \
             tc.tile_pool(name="attn_sb", bufs=2) as attn_sb, \
             tc.tile_pool(name="normp", bufs=2) as normp, \
             tc.tile_pool(name="outst", bufs=2) as outst, \
             tc.tile_pool(name="xf", bufs=1) as xf, \
             tc.tile_pool(name="psum", bufs=1, space="PSUM") as psum:

            # ---- constants & resident activations ----
            wq_s = consts.tile([128, 8, GDIM], bf16)
            wk_s = consts.tile([128, 8, GDIM], bf16)
            wv_s = consts.tile([128, 8, GDIM], bf16)
            wo_s = consts.tile([128, 2, D], bf16)
            cos_s = consts.tile([128, S], bf16)
            sin_s = consts.tile([128, S], bf16)
            heat = consts.tile([128, 128], bf16)

            xq = xf.tile([128, 8, S], bf16, tag="xq", name="xq")
            xk = xf.tile([128, 8, S], bf16, tag="xk", name="xk")

            # ---- prologue DMA schedule ----
            # All queues share the same 16 SDMA engines (~358 GB/s total), so
            # what matters is DMA SIZE (1MB ~ 78% efficiency vs 32% at 64KB).
            # x moves in ~1MB 4-tile chunks, first halves (which feed the w=0
            # projection waves and qc0/qc1 attention) first.
            nc.scalar.dma_start(out=cos_s, in_=dcos)
            nc.scalar.dma_start(out=sin_s, in_=dsin)
            nc.sync.dma_start(out=wq_s, in_=dwq)
            nc.gpsimd.dma_start(out=wk_s, in_=dwk)
            lo, hi = slice(0, 4), slice(4, 8)
            # x in [p, half, t, 1024] layout: each (tensor, half, t-quad) DMA
            # reads 8KB-contiguous per partition
            nc.sync.dma_start(out=xq[:, lo, 0:1024], in_=dqT[:, 0, lo, :])
            nc.gpsimd.dma_start(out=xq[:, hi, 0:1024], in_=dqT[:, 0, hi, :])
            nc.sync.dma_start(out=xk[:, lo, 0:1024], in_=dkT[:, 0, lo, :])
            nc.gpsimd.dma_start(out=xk[:, hi, 0:1024], in_=dkT[:, 0, hi, :])
            nc.scalar.dma_start(out=wv_s, in_=dwv)
            nc.gpsimd.dma_start(out=wo_s, in_=dwo)

            # V activations stream in per q-chunk (vf tiles) instead of a
            # resident 4MB buffer: takes them off the prologue critical path.
            def make_vf(qc):
                vft = xf.tile([128, 8, 512], bf16, tag="vf", bufs=2,
                              name=f"vf{qc}")
                eng = nc.scalar if qc < 2 else nc.gpsimd
                eng.dma_start(out=vft, in_=dvT[qc])
                return vft

            vfs = {0: make_vf(0), 1: make_vf(1)}

            # second x halves (feed the deferred w=1 projection waves)
            nc.sync.dma_start(out=xq[:, lo, 1024:2048], in_=dqT[:, 1, lo, :])
            nc.gpsimd.dma_start(out=xq[:, hi, 1024:2048], in_=dqT[:, 1, hi, :])
            nc.sync.dma_start(out=xk[:, lo, 1024:2048], in_=dkT[:, 1, lo, :])
            nc.gpsimd.dma_start(out=xk[:, hi, 1024:2048], in_=dkT[:, 1, hi, :])

            # PE heater: keep HAM busy during initial DMA wait
            nc.vector.memset(heat, 0.0)
            ones_f = consts.tile([1, DK], f32)
            nc.vector.memset(ones_f, 1.0)
            ones_r = consts.tile([1, DK], f32r)
            nc.vector.tensor_copy(out=ones_r, in_=ones_f)

            QT = [persist.tile([128, S], bf16, name=f"QT{m}") for m in range(2)]
            KTt = [persist.tile([128, S], bf16, name=f"KTt{m}") for m in range(2)]
            Vt = persist.tile([128, KT, HPC, DK + 1], bf16)
            cxT = [persist.tile([128, S], bf16, name=f"cxT{m}") for m in range(2)]
            nc.vector.memset(Vt[:, :, :, DK:DK + 1], 1.0)

            hps = psum.tile([128, 512], f32, tag="sml", bufs=2, name="hps")
            for i in range(12):
                nc.tensor.matmul(hps[:, 0:128], lhsT=heat, rhs=heat,
                                 start=True, stop=True)

            def rope_w(pp, dst, w):
                # pp: 2 psum [128,512] proj.T chunks for cols 1024w:1024w+1024.
                # sin_s rows are host-pre-swapped so each swap-mul reads both
                # inputs at the same base partition (DVE requirement); only
                # the output lands at the swapped 32-block.
                cols = slice(1024 * w, 1024 * w + 1024)
                qb = ropep.tile([128, 1024], bf16, tag="qb", name="qb")
                for j in range(2):
                    nc.scalar.copy(out=qb[:, 512 * j:512 * j + 512], in_=pp[j])
                ss = ropep.tile([128, 1024], bf16, tag="ss", name="ss")
                for base in (0, 64):
                    nc.vector.tensor_mul(out=ss[base:base + 32, :],
                                         in0=qb[base + 32:base + 64, :],
                                         in1=sin_s[base + 32:base + 64, cols])
                    nc.vector.tensor_mul(out=ss[base + 32:base + 64, :],
                                         in0=qb[base:base + 32, :],
                                         in1=sin_s[base:base + 32, cols])
                cc = ropep.tile([128, 1024], bf16, tag="cc", name="cc")
                nc.vector.tensor_mul(out=cc, in0=qb, in1=cos_s[:, cols])
                nc.vector.tensor_add(out=dst[:, cols], in0=cc, in1=ss)

            def proj_block(w):
                # Q/K projections for output cols 1024w:1024w+1024, both m.
                for m in range(2):
                    for which, wsrc, xsrc, dsts in (("q", wq_s, xq, QT),
                                                    ("k", wk_s, xk, KTt)):
                        pp = [psum.tile([128, 512], f32, tag="sml", bufs=2,
                                        name=f"pp{which}{m}{w}{n}")
                              for n in (2 * w, 2 * w + 1)]
                        for t in range(8):
                            for j, n in enumerate((2 * w, 2 * w + 1)):
                                nc.tensor.matmul(
                                    pp[j],
                                    lhsT=wsrc[:, t, 128 * m:128 * m + 128],
                                    rhs=xsrc[:, t, 512 * n:512 * n + 512],
                                    start=(t == 0), stop=(t == 7))
                        rope_w(pp, dsts[m], w)

            def vproj(qc):
                vft = vfs[qc]
                for jj in range(4):
                    tt = 4 * qc + jj
                    pv = psum.tile([128, GDIM], f32, tag="sml", bufs=2,
                                   name="pv")
                    for t in range(8):
                        nc.tensor.matmul(
                            pv,
                            lhsT=vft[:, t, 128 * jj:128 * jj + 128],
                            rhs=wv_s[:, t, :],
                            start=(t == 0), stop=(t == 7))
                    nc.vector.tensor_copy(
                        out=Vt[:, tt, :, 0:DK],
                        in_=pv.rearrange("p (h d) -> p h d", h=HPC))


            def load_eb(m, qc, gi):
                # fp8(e3m4)-packed bias, cast to bf16 in-flight by the SWDGE
                gn, off = EB_OFF[(m, qc, gi)]
                ebt2 = attn_sb.tile([128, 2, 2048], bf16, tag="ebt",
                                    bufs=2, name="ebt2")
                nc.gpsimd.dma_start(
                    out=ebt2[:, :, 0:gn],
                    in_=deb[off:off + 2 * 128 * gn].rearrange(
                        "(a p n) -> p a n", a=2, p=128))
                return ebt2

            # prefetch qc0's bias (one grp per m) ahead of the x second halves
            eb_pre = {m: load_eb(m, 0, 0) for m in range(2)}

            def attn_qc(qc):
                for m in range(2):          # head pair (2m, 2m+1)
                    pcx = [psum.tile([DK + 1, 512], f32, tag="pcx", bufs=2,
                                     name=f"pcx{a}") for a in range(2)]
                    last_kt = 4 * qc + 3
                    for gi, grp in enumerate(_quads(qc)):
                        gn = sum(n for kt, n, q0 in grp)
                        praw = [attn_sb.tile([128, 2048], bf16, tag=f"praw{a}",
                                             bufs=2, name=f"praw{a}")
                                for a in range(2)]
                        if qc == 0 and gi == 0:
                            ebt2 = eb_pre[m]
                        else:
                            ebt2 = load_eb(m, qc, gi)
                        goff = 0
                        for pi in range(0, 4, 2):
                            pair = grp[pi:pi + 2]
                            pss = [psum.tile([128, 1024], f32, tag="pss",
                                             bufs=2, name=f"pss{a}")
                                   for a in range(2)]
                            soff = 0
                            for kt, n, q0 in pair:
                                # a-inner: the two row-tiled matmuls run
                                # concurrently, hiding each other's LDWEIGHTS
                                for a in range(2):
                                    nc.tensor.matmul(
                                        pss[a][:, soff:soff + n],
                                        lhsT=KTt[m][64 * a:64 * a + DK,
                                                    128 * kt:128 * kt + 128],
                                        rhs=QT[m][64 * a:64 * a + DK,
                                                  q0:q0 + n],
                                        start=True, stop=True,
                                        tile_position=(64 * a, 0))
                                soff += n
                            for a in range(2):
                                nc.scalar.activation(
                                    out=praw[a][:, goff:goff + soff],
                                    in_=pss[a][:, 0:soff],
                                    func=mybir.ActivationFunctionType.Exp)
                            goff += soff
                        for a in range(2):
                            nc.vector.tensor_mul(out=praw[a][:, 0:gn],
                                                 in0=praw[a][:, 0:gn],
                                                 in1=ebt2[:, a, 0:gn])
                        goff = 0
                        for kt, n, q0 in grp:
                            co = q0 - 512 * qc
                            for a in range(2):
                                nc.tensor.matmul(
                                    pcx[a][:, co:co + n],
                                    lhsT=Vt[:, kt, 2 * m + a, :],
                                    rhs=praw[a][:, goff:goff + n],
                                    start=(kt == 0), stop=(kt == last_kt))
                            goff += n
                    # normalize: per-head broadcast of the denominator row via
                    # a K=1 matmul, reciprocal on the broadcast, scale ctx.
                    for a in range(2):
                        lone = normp.tile([1, 512], f32r, tag="lone",
                                          name="lone")
                        nc.vector.tensor_copy(out=lone,
                                              in_=pcx[a][DK:DK + 1, :])
                        pb = psum.tile([128, 512], f32, tag="sml", bufs=2,
                                       name="pb")
                        nc.tensor.matmul(pb[0:DK, :], lhsT=ones_r, rhs=lone,
                                         start=True, stop=True)
                        rb = normp.tile([DK, 512], f32, tag="rb", name="rb")
                        nc.vector.reciprocal_approx_fast(out=rb,
                                                         in_=pb[0:DK, :])
                        nc.vector.tensor_mul(
                            out=cxT[m][64 * a:64 * a + DK,
                                       512 * qc:512 * qc + 512],
                            in0=pcx[a][0:DK, :],
                            in1=rb)

                # output projection for this qc's 4 token tiles
                for tt in range(4 * qc, 4 * qc + 4):
                    po = [psum.tile([128, 512], f32, tag="sml", bufs=2,
                                    name=f"po{e}") for e in range(2)]
                    for m in range(2):
                        for e in range(2):
                            nc.tensor.matmul(
                                po[e],
                                lhsT=cxT[m][:, 128 * tt:128 * tt + 128],
                                rhs=wo_s[:, m, 512 * e:512 * e + 512],
                                start=(m == 0), stop=(m == 1))
                    ost = outst.tile([128, D], f32, tag="ost")
                    nc.scalar.copy(out=ost[:, 0:512], in_=po[0])
                    nc.vector.tensor_copy(out=ost[:, 512:1024], in_=po[1])
                    nc.sync.dma_start(out=dout[128 * tt:128 * tt + 128, :],
                                      in_=ost)

            # ---- emission: w0 projections -> qc0 -> w1 projections -> qc1.. ----
            proj_block(0)
            vproj(0)
            attn_qc(0)
            proj_block(1)
            vfs[2] = make_vf(2)
            vproj(1)
            attn_qc(1)
            vfs[3] = make_vf(3)
            vproj(2)
            attn_qc(2)
            vproj(3)
            attn_qc(3)

    nc.compile()
    return nc


def _get_program():
    global _PROGRAM
    if _PROGRAM is None:
        _PROGRAM = _build_program()
    return _PROGRAM


def _rope_tables():
    half = DK // 2
    inv_freq = 1.0 / (10000.0 ** (np.arange(half, dtype=np.float64) / half))
    ang = np.arange(S, dtype=np.float64)[:, None] * inv_freq[None, :]  # [S, 32]
    cos = np.cos(ang).T  # [32, S]
    sin = np.sin(ang).T
    cos64 = np.concatenate([cos, cos], axis=0)            # [64, S]
    # signed for rotate-half AND half-swapped: the device reads the sin row at
    # the SOURCE partition of each swap-mul (row p holds the coefficient that
    # multiplies data row p before it lands at the swapped position).
    sin64 = np.concatenate([sin, -sin], axis=0)
    cosT = np.tile(cos64, (2, 1)).astype(BF16)            # [128, S]
    sinT = np.tile(sin64, (2, 1)).astype(BF16)
    return np.ascontiguousarray(cosT), np.ascontiguousarray(sinT)


def _eb_layout():
    """Yields (m, qc, gi, grp, gn, off): the packed-bias block layout. Each
    block holds BOTH heads of the m-pair ([a, 128, gn] contiguous) so the
    device loads it with a single ~1MB DMA."""
    off = 0
    for m in range(2):
        for qc in range(QC):
            for gi, grp in enumerate(_quads(qc)):
                gn = sum(n for kt, n, q0 in grp)
                yield m, qc, gi, grp, gn, off
                off += 2 * 128 * gn


EB_OFF = {(m, qc, gi): (gn, off)
          for m, qc, gi, grp, gn, off in _eb_layout()}


def _pack_ebias(bias_g):
    """bias_g: [HPC, S, S] f32 (this group's heads). Returns packed 1D fp8
    (e3m4), one contiguous [a, 128, gn] block per (m, qc, kt-quad)."""
    out = np.empty(EB_TOTAL, dtype=FP8E3)
    tri = np.triu(np.ones((128, 128), dtype=np.float32))
    for m, qc, gi, grp, gn, off in _eb_layout():
        for a in range(2):
            h = 2 * m + a
            blks = []
            for kt, n, q0 in grp:
                blk = np.exp(
                    bias_g[h, q0:q0 + n, 128 * kt:128 * kt + 128]
                    .astype(np.float64)).T.astype(np.float32)  # [128, n]
                if kt // 4 == qc:
                    blk[:, 0:128] *= tri
                blks.append(blk)
            wide = np.concatenate(blks, axis=1).astype(FP8E3)  # [128, gn]
            base = off + a * 128 * gn
            out[base:base + 128 * gn] = wide.reshape(-1)
    return out


def _prep_inputs(query, key, value, rel_pos_bias, Wq, Wk, Wv, Wo_w):
    cosT, sinT = _rope_tables()
    xT = {}
    for nm, x in (("q", query), ("k", key)):
        for b in range(B):
            t = np.ascontiguousarray(
                x[b].T.reshape(8, 128, 2, 1024).transpose(1, 2, 0, 3)
            ).astype(BF16)  # [128, half, t, 1024]
            xT[(nm, b)] = t
    for b in range(B):
        t = np.ascontiguousarray(
            value[b].T.reshape(8, 128, 4, 512).transpose(2, 1, 0, 3)
        ).astype(BF16)  # [qc, 128, t, 512]
        xT[("v", b)] = t
    wqs, wks, wvs, wos, ebs = {}, {}, {}, {}, {}
    for g in range(4):
        sl = slice(GDIM * g, GDIM * (g + 1))
        wqs[g] = np.ascontiguousarray(
            (Wq[sl, :] / SCALE).T.reshape(8, 128, GDIM).transpose(1, 0, 2)
        ).astype(BF16)
        wks[g] = np.ascontiguousarray(
            Wk[sl, :].T.reshape(8, 128, GDIM).transpose(1, 0, 2)).astype(BF16)
        wvs[g] = np.ascontiguousarray(
            Wv[sl, :].T.reshape(8, 128, GDIM).transpose(1, 0, 2)).astype(BF16)
        wos[g] = np.ascontiguousarray(
            Wo_w[:, sl].T.reshape(2, 128, D).transpose(1, 0, 2)).astype(BF16)
        ebs[g] = _pack_ebias(rel_pos_bias[0, HPC * g:HPC * (g + 1)])
    in_maps = []
    for c in range(N_CORES):
        b, g = c // 4, c % 4
        in_maps.append({
            "qT": xT[("q", b)], "kT": xT[("k", b)], "vT": xT[("v", b)],
            "wq": wqs[g], "wk": wks[g], "wv": wvs[g], "wo": wos[g],
            "eb": ebs[g], "cosT": cosT, "sinT": sinT,
        })
    return in_maps


def _run(query, key, value, rel_pos_bias, Wq, Wk, Wv, Wo_w, Wo_b, trace=False,
         **trace_kwargs):
    nc = _get_program()
    in_maps = _prep_inputs(query, key, value, rel_pos_bias, Wq, Wk, Wv, Wo_w)
    res = run_bass_kernel_spmd(nc, in_maps, core_ids=list(range(N_CORES)),
                               trace=trace, **trace_kwargs)
    out = np.empty((B, S, D), dtype=np.float32)
    for b in range(B):
        acc = res.results[4 * b]["out"].astype(np.float32)
        for g in range(1, 4):
            acc = acc + res.results[4 * b + g]["out"]
        out[b] = acc + Wo_b[None, :]
    return out, res


def _cpu_fallback(query, key, value, mask, rel_pos_bias, Wq, Wk, Wv, Wo_w, Wo_b):
    def rope_np(x):
        half = DK // 2
        inv_freq = 1.0 / (10000.0 ** (np.arange(half, dtype=np.float32) / half))
        ang = np.arange(S, dtype=np.float32)[:, None] * inv_freq[None, :]
        cos = np.concatenate([np.cos(ang), np.cos(ang)], axis=-1)[None, None]
        sin = np.concatenate([np.sin(ang), np.sin(ang)], axis=-1)[None, None]
        x1, x2 = x[..., :half], x[..., half:]
        rot = np.concatenate([-x2, x1], axis=-1)
        return x * cos + rot * sin

    q = np.einsum('bsd,ed->bse', query, Wq).reshape(B, S, H, DK).transpose(0, 2, 1, 3)
    k = np.einsum('bsd,ed->bse', key, Wk).reshape(B, S, H, DK).transpose(0, 2, 1, 3)
    v = np.einsum('bsd,ed->bse', value, Wv).reshape(B, S, H, DK).transpose(0, 2, 1, 3)
    q, k = rope_np(q), rope_np(k)
    sc = np.einsum('bhqd,bhkd->bhqk', q, k) / SCALE + rel_pos_bias
    sc = np.where(mask, sc, -np.inf)
    sc = sc - sc.max(axis=-1, keepdims=True)
    e = np.exp(sc)
    attn = e / e.sum(axis=-1, keepdims=True)
    ctx = np.einsum('bhqk,bhkd->bhqd', attn, v)
    ctx = ctx.transpose(0, 2, 1, 3).reshape(B, S, D)
    return (np.einsum('bsd,ed->bse', ctx, Wo_w) + Wo_b).astype(np.float32)


def kernel(query, key, value, mask, rel_pos_bias, Wq, Wk, Wv, Wo_w, Wo_b):
    query = np.asarray(query, dtype=np.float32)
    key = np.asarray(key, dtype=np.float32)
    value = np.asarray(value, dtype=np.float32)
    mask = np.asarray(mask)
    rel_pos_bias = np.asarray(rel_pos_bias, dtype=np.float32)
    Wq = np.asarray(Wq, dtype=np.float32)
    Wk = np.asarray(Wk, dtype=np.float32)
    Wv = np.asarray(Wv, dtype=np.float32)
    Wo_w = np.asarray(Wo_w, dtype=np.float32)
    Wo_b = np.asarray(Wo_b, dtype=np.float32)

    if not np.array_equal(mask.reshape(S, S),
                          np.tril(np.ones((S, S), dtype=bool))):
        return _cpu_fallback(query, key, value, mask, rel_pos_bias,
                             Wq, Wk, Wv, Wo_w, Wo_b)

    out, _ = _run(query, key, value, rel_pos_bias, Wq, Wk, Wv, Wo_w, Wo_b)
    return out


# revision 16
# speedup vs baseline: 1.0048x; 1.0048x over previous
"""Multi-head attention (B=2, S=2048, D=1024, H=16, causal + rel-pos-bias + RoPE)
on 8 Trainium2 NeuronCores.

Sharding: core c handles batch c//4 and head-group c%4 (4 heads = 256 model dims).
Each core computes its heads' Q/K/V projections (column-sharded weights), RoPE,
causal attention with relative position bias, and a partial output projection
(row-sharded Wo). Host sums the 4 partials per batch and adds Wo_b.
"""

import math

import numpy as np
import ml_dtypes

import concourse.bass as bass
import concourse.mybir as mybir
import concourse.tile as tile
from concourse import bacc
from concourse.bass_utils import run_bass_kernel_spmd

BF16 = ml_dtypes.bfloat16
FP8E3 = ml_dtypes.float8_e3m4

B, S, D, H = 2, 2048, 1024, 16
DK = 64
SCALE = math.sqrt(DK)
HPC = 4          # heads per core
GDIM = HPC * DK  # 256 model dims per core
N_CORES = 8
KT = S // 128    # 16 k-tiles
QC = S // 512    # 4 q-chunks

f32 = mybir.dt.float32
f32r = mybir.dt.float32r
bf16 = mybir.dt.bfloat16


def _sched():
    """Attention tile schedule, shared by host bias packer and device builder.

    Yields (h, qc, kt, n, q0): head-local index, q-chunk, k-tile, the valid
    column count and starting q of the S^T tile [128 k, n q]."""
    for h in range(HPC):
        for qc in range(QC):
            for kt in range(4 * qc + 4):
                if kt // 4 == qc:  # diagonal-crossing tile
                    n = 512 - 128 * (kt % 4)
                    q0 = 128 * kt
                else:
                    n = 512
                    q0 = 512 * qc
                yield h, qc, kt, n, q0


EB_PER_HEAD = sum(128 * n for h, qc, kt, n, q0 in _sched()) // HPC
EB_TOTAL = EB_PER_HEAD * HPC

_PROGRAM = None


def _quads(qc):
    """kt quad-groups for one (h, qc) chunk: list of [(kt,n,q0)...]."""
    kts = list(range(4 * qc + 4))
    out = []
    for i in range(0, len(kts), 4):
        grp = []
        for kt in kts[i:i + 4]:
            if kt // 4 == qc:
                n = 512 - 128 * (kt % 4)
                q0 = 128 * kt
            else:
                n = 512
                q0 = 512 * qc
            grp.append((kt, n, q0))
        out.append(grp)
    return out


def _build_program():
    nc = bacc.Bacc("TRN2", target_bir_lowering=False, debug=False)

    dqT = nc.dram_tensor("qT", [128, 2, 8, 1024], bf16,
                         kind="ExternalInput").ap()
    dkT = nc.dram_tensor("kT", [128, 2, 8, 1024], bf16,
                         kind="ExternalInput").ap()
    dvT = nc.dram_tensor("vT", [4, 128, 8, 512], bf16,
                         kind="ExternalInput").ap()
    dwq = nc.dram_tensor("wq", [128, 8, GDIM], bf16,
                         kind="ExternalInput").ap()
    dwk = nc.dram_tensor("wk", [128, 8, GDIM], bf16,
                         kind="ExternalInput").ap()
    dwv = nc.dram_tensor("wv", [128, 8, GDIM], bf16,
                         kind="ExternalInput").ap()
    dwo = nc.dram_tensor("wo", [128, 2, D], bf16, kind="ExternalInput").ap()
    deb = nc.dram_tensor("eb", [EB_TOTAL], mybir.dt.float8e3,
                         kind="ExternalInput").ap()
    dcos = nc.dram_tensor("cosT", [128, S], bf16, kind="ExternalInput").ap()
    dsin = nc.dram_tensor("sinT", [128, S], bf16, kind="ExternalInput").ap()
    dout = nc.dram_tensor("out", [S, D], f32, kind="ExternalOutput").ap()

    with tile.TileContext(nc) as tc:
        with tc.tile_pool(name="consts", bufs=1) as consts, \
             tc.tile_pool(name="persist", bufs=1) as persist, \
             tc.tile_pool(name="ropep", bufs=2) as ropep, \
             tc.tile_pool(name="attn_sb", bufs=2) as attn_sb, \
             tc.tile_pool(name="normp", bufs=2) as normp, \
             tc.tile_pool(name="outst", bufs=2) as outst, \
             tc.tile_pool(name="xf", bufs=1) as xf, \
             tc.tile_pool(name="psum", bufs=1, space="PSUM") as psum:

            # ---- constants & resident activations ----
            wq_s = consts.tile([128, 8, GDIM], bf16)
            wk_s = consts.tile([128, 8, GDIM], bf16)
            wv_s = consts.tile([128, 8, GDIM], bf16)
            wo_s = consts.tile([128, 2, D], bf16)
            cos_s = consts.tile([128, S], bf16)
            sin_s = consts.tile([128, S], bf16)
            heat = consts.tile([128, 128], bf16)

            # x quadrants as separate tiles: the 4 DMAs per tensor would
            # otherwise serialize on whole-tile write dependencies
            xp = {}
            for which in ("q", "k"):
                for tq in range(2):
                    for h in range(2):
                        xp[(which, tq, h)] = xf.tile(
                            [128, 4, 1024], bf16, tag=f"x{which}{tq}{h}",
                            name=f"x{which}{tq}{h}")

            # ---- prologue DMA schedule ----
            # All queues share the same 16 SDMA engines (~358 GB/s total), so
            # what matters is DMA SIZE (1MB ~ 78% efficiency vs 32% at 64KB).
            # x moves in ~1MB 4-tile chunks, first halves (which feed the w=0
            # projection waves and qc0/qc1 attention) first.
            nc.scalar.dma_start(out=cos_s, in_=dcos)
            nc.scalar.dma_start(out=sin_s, in_=dsin)
            nc.sync.dma_start(out=wq_s, in_=dwq)
            nc.gpsimd.dma_start(out=wk_s, in_=dwk)
            lo, hi = slice(0, 4), slice(4, 8)
            # x in [p, half, t, 1024] layout: each (tensor, half, t-quad) DMA
            # reads 8KB-contiguous per partition
            nc.sync.dma_start(out=xp[("q", 0, 0)], in_=dqT[:, 0, lo, :])
            nc.gpsimd.dma_start(out=xp[("q", 1, 0)], in_=dqT[:, 0, hi, :])
            nc.sync.dma_start(out=xp[("k", 0, 0)], in_=dkT[:, 0, lo, :])
            nc.gpsimd.dma_start(out=xp[("k", 1, 0)], in_=dkT[:, 0, hi, :])
            nc.scalar.dma_start(out=wv_s, in_=dwv)
            nc.gpsimd.dma_start(out=wo_s, in_=dwo)

            # V activations stream in per q-chunk (vf tiles) instead of a
            # resident 4MB buffer: takes them off the prologue critical path.
            def make_vf(qc):
                vft = xf.tile([128, 8, 512], bf16, tag="vf", bufs=2,
                              name=f"vf{qc}")
                eng = nc.scalar if qc < 2 else nc.gpsimd
                eng.dma_start(out=vft, in_=dvT[qc])
                return vft

            vfs = {0: make_vf(0), 1: make_vf(1)}

            # second x halves (feed the deferred w=1 projection waves)
            nc.sync.dma_start(out=xp[("q", 0, 1)], in_=dqT[:, 1, lo, :])
            nc.gpsimd.dma_start(out=xp[("q", 1, 1)], in_=dqT[:, 1, hi, :])
            nc.sync.dma_start(out=xp[("k", 0, 1)], in_=dkT[:, 1, lo, :])
            nc.gpsimd.dma_start(out=xp[("k", 1, 1)], in_=dkT[:, 1, hi, :])

            # PE heater: keep HAM busy during initial DMA wait
            nc.vector.memset(heat, 0.0)
            ones_f = consts.tile([1, DK], f32)
            nc.vector.memset(ones_f, 1.0)
            ones_r = consts.tile([1, DK], f32r)
            nc.vector.tensor_copy(out=ones_r, in_=ones_f)

            QT = [persist.tile([128, S], bf16, name=f"QT{m}") for m in range(2)]
            KTt = [persist.tile([128, S], bf16, name=f"KTt{m}") for m in range(2)]
            Vt = persist.tile([128, KT, HPC, DK + 1], bf16)
            cxT = [persist.tile([128, S], bf16, name=f"cxT{m}") for m in range(2)]
            nc.vector.memset(Vt[:, :, :, DK:DK + 1], 1.0)

            hps = psum.tile([128, 512], f32, tag="sml", bufs=2, name="hps")
            for i in range(12):
                nc.tensor.matmul(hps[:, 0:128], lhsT=heat, rhs=heat,
                                 start=True, stop=True)

            def rope_w(pp, dst, w):
                # pp: 2 psum [128,512] proj.T chunks for cols 1024w:1024w+1024.
                # sin_s rows are host-pre-swapped so each swap-mul reads both
                # inputs at the same base partition (DVE requirement); only
                # the output lands at the swapped 32-block.
                cols = slice(1024 * w, 1024 * w + 1024)
                qb = ropep.tile([128, 1024], bf16, tag="qb", name="qb")
                for j in range(2):
                    nc.scalar.copy(out=qb[:, 512 * j:512 * j + 512], in_=pp[j])
                ss = ropep.tile([128, 1024], bf16, tag="ss", name="ss")
                for base in (0, 64):
                    nc.vector.tensor_mul(out=ss[base:base + 32, :],
                                         in0=qb[base + 32:base + 64, :],
                                         in1=sin_s[base + 32:base + 64, cols])
                    nc.vector.tensor_mul(out=ss[base + 32:base + 64, :],
                                         in0=qb[base:base + 32, :],
                                         in1=sin_s[base:base + 32, cols])
                cc = ropep.tile([128, 1024], bf16, tag="cc", name="cc")
                nc.vector.tensor_mul(out=cc, in0=qb, in1=cos_s[:, cols])
                nc.vector.tensor_add(out=dst[:, cols], in0=cc, in1=ss)

            def proj_block(w):
                # Q/K projections for output cols 1024w:1024w+1024, both m.
                for m in range(2):
                    for which, wsrc, dsts in (("q", wq_s, QT),
                                              ("k", wk_s, KTt)):
                        pp = [psum.tile([128, 512], f32, tag="sml", bufs=2,
                                        name=f"pp{which}{m}{w}{n}")
                              for n in (2 * w, 2 * w + 1)]
                        for t in range(8):
                            xt = xp[(which, t // 4, w)]
                            for j in range(2):
                                nc.tensor.matmul(
                                    pp[j],
                                    lhsT=wsrc[:, t, 128 * m:128 * m + 128],
                                    rhs=xt[:, t % 4, 512 * j:512 * j + 512],
                                    start=(t == 0), stop=(t == 7))
                        rope_w(pp, dsts[m], w)

            def vproj(qc):
                vft = vfs[qc]
                for jj in range(4):
                    tt = 4 * qc + jj
                    pv = psum.tile([128, GDIM], f32, tag="sml", bufs=2,
                                   name="pv")
                    for t in range(8):
                        nc.tensor.matmul(
                            pv,
                            lhsT=vft[:, t, 128 * jj:128 * jj + 128],
                            rhs=wv_s[:, t, :],
                            start=(t == 0), stop=(t == 7))
                    nc.vector.tensor_copy(
                        out=Vt[:, tt, :, 0:DK],
                        in_=pv.rearrange("p (h d) -> p h d", h=HPC))


            def load_eb(m, qc, gi):
                # fp8(e3m4)-packed bias, cast to bf16 in-flight by the SWDGE
                gn, off = EB_OFF[(m, qc, gi)]
                ebt2 = attn_sb.tile([128, 2, 2048], bf16, tag="ebt",
                                    bufs=2, name="ebt2")
                nc.gpsimd.dma_start(
                    out=ebt2[:, :, 0:gn],
                    in_=deb[off:off + 2 * 128 * gn].rearrange(
                        "(a p n) -> p a n", a=2, p=128))
                return ebt2

            # prefetch qc0's bias (one grp per m) ahead of the x second halves
            eb_pre = {m: load_eb(m, 0, 0) for m in range(2)}

            def attn_qc(qc):
                for m in range(2):          # head pair (2m, 2m+1)
                    pcx = [psum.tile([DK + 1, 512], f32, tag="pcx", bufs=2,
                                     name=f"pcx{a}") for a in range(2)]
                    last_kt = 4 * qc + 3
                    for gi, grp in enumerate(_quads(qc)):
                        gn = sum(n for kt, n, q0 in grp)
                        praw = [attn_sb.tile([128, 2048], bf16, tag=f"praw{a}",
                                             bufs=2, name=f"praw{a}")
                                for a in range(2)]
                        if qc == 0 and gi == 0:
                            ebt2 = eb_pre[m]
                        else:
                            ebt2 = load_eb(m, qc, gi)
                        goff = 0
                        for pi in range(0, 4, 2):
                            pair = grp[pi:pi + 2]
                            pss = [psum.tile([128, 1024], f32, tag="pss",
                                             bufs=2, name=f"pss{a}")
                                   for a in range(2)]
                            soff = 0
                            for kt, n, q0 in pair:
                                # a-inner: the two row-tiled matmuls run
                                # concurrently, hiding each other's LDWEIGHTS
                                for a in range(2):
                                    nc.tensor.matmul(
                                        pss[a][:, soff:soff + n],
                                        lhsT=KTt[m][64 * a:64 * a + DK,
                                                    128 * kt:128 * kt + 128],
                                        rhs=QT[m][64 * a:64 * a + DK,
                                                  q0:q0 + n],
                                        start=True, stop=True,
                                        tile_position=(64 * a, 0))
                                soff += n
                            for a in range(2):
                                nc.scalar.activation(
                                    out=praw[a][:, goff:goff + soff],
                                    in_=pss[a][:, 0:soff],
                                    func=mybir.ActivationFunctionType.Exp)
                            goff += soff
                        for a in range(2):
                            nc.vector.tensor_mul(out=praw[a][:, 0:gn],
                                                 in0=praw[a][:, 0:gn],
                                                 in1=ebt2[:, a, 0:gn])
                        goff = 0
                        for kt, n, q0 in grp:
                            co = q0 - 512 * qc
                            for a in range(2):
                                nc.tensor.matmul(
                                    pcx[a][:, co:co + n],
                                    lhsT=Vt[:, kt, 2 * m + a, :],
                                    rhs=praw[a][:, goff:goff + n],
                                    start=(kt == 0), stop=(kt == last_kt))
                            goff += n
                    # normalize: per-head broadcast of the denominator row via
                    # a K=1 matmul, reciprocal on the broadcast, scale ctx.
                    for a in range(2):
                        lone = normp.tile([1, 512], f32r, tag="lone",
                                          name="lone")
                        nc.vector.tensor_copy(out=lone,
                                              in_=pcx[a][DK:DK + 1, :])
                        pb = psum.tile([128, 512], f32, tag="sml", bufs=2,
                                       name="pb")
                        nc.tensor.matmul(pb[0:DK, :], lhsT=ones_r, rhs=lone,
                                         start=True, stop=True)
                        rb = normp.tile([DK, 512], f32, tag="rb", name="rb")
                        nc.vector.reciprocal_approx_fast(out=rb,
                                                         in_=pb[0:DK, :])
                        nc.vector.tensor_mul(
                            out=cxT[m][64 * a:64 * a + DK,
                                       512 * qc:512 * qc + 512],
                            in0=pcx[a][0:DK, :],
                            in1=rb)

                # output projection for this qc's 4 token tiles
                for tt in range(4 * qc, 4 * qc + 4):
                    po = [psum.tile([128, 512], f32, tag="sml", bufs=2,
                                    name=f"po{e}") for e in range(2)]
                    for m in range(2):
                        for e in range(2):
                            nc.tensor.matmul(
                                po[e],
                                lhsT=cxT[m][:, 128 * tt:128 * tt + 128],
                                rhs=wo_s[:, m, 512 * e:512 * e + 512],
                                start=(m == 0), stop=(m == 1))
                    ost = outst.tile([128, D], f32, tag="ost")
                    nc.scalar.copy(out=ost[:, 0:512], in_=po[0])
                    nc.vector.tensor_copy(out=ost[:, 512:1024], in_=po[1])
                    nc.sync.dma_start(out=dout[128 * tt:128 * tt + 128, :],
                                      in_=ost)

            # ---- emission: w0 projections -> qc0 -> w1 projections -> qc1.. ----
            proj_block(0)
            vproj(0)
            attn_qc(0)
            proj_block(1)
            vfs[2] = make_vf(2)
            vproj(1)
            attn_qc(1)
            vfs[3] = make_vf(3)
            vproj(2)
            attn_qc(2)
            vproj(3)
            attn_qc(3)

    nc.compile()
    return nc


def _get_program():
    global _PROGRAM
    if _PROGRAM is None:
        _PROGRAM = _build_program()
    return _PROGRAM


def _rope_tables():
    half = DK // 2
    inv_freq = 1.0 / (10000.0 ** (np.arange(half, dtype=np.float64) / half))
    ang = np.arange(S, dtype=np.float64)[:, None] * inv_freq[None, :]  # [S, 32]
    cos = np.cos(ang).T  # [32, S]
    sin = np.sin(ang).T
    cos64 = np.concatenate([cos, cos], axis=0)            # [64, S]
    # signed for rotate-half AND half-swapped: the device reads the sin row at
    # the SOURCE partition of each swap-mul (row p holds the coefficient that
    # multiplies data row p before it lands at the swapped position).
    sin64 = np.concatenate([sin, -sin], axis=0)
    cosT = np.tile(cos64, (2, 1)).astype(BF16)            # [128, S]
    sinT = np.tile(sin64, (2, 1)).astype(BF16)
    return np.ascontiguousarray(cosT), np.ascontiguousarray(sinT)


def _eb_layout():
    """Yields (m, qc, gi, grp, gn, off): the packed-bias block layout. Each
    block holds BOTH heads of the m-pair ([a, 128, gn] contiguous) so the
    device loads it with a single ~1MB DMA."""
    off = 0
    for m in range(2):
        for qc in range(QC):
            for gi, grp in enumerate(_quads(qc)):
                gn = sum(n for kt, n, q0 in grp)
                yield m, qc, gi, grp, gn, off
                off += 2 * 128 * gn


EB_OFF = {(m, qc, gi): (gn, off)
          for m, qc, gi, grp, gn, off in _eb_layout()}


def _pack_ebias(bias_g):
    """bias_g: [HPC, S, S] f32 (this group's heads). Returns packed 1D fp8
    (e3m4), one contiguous [a, 128, gn] block per (m, qc, kt-quad)."""
    out = np.empty(EB_TOTAL, dtype=FP8E3)
    tri = np.triu(np.ones((128, 128), dtype=np.float32))
    for m, qc, gi, grp, gn, off in _eb_layout():
        for a in range(2):
            h = 2 * m + a
            blks = []
            for kt, n, q0 in grp:
                blk = np.exp(
                    bias_g[h, q0:q0 + n, 128 * kt:128 * kt + 128]
                    .astype(np.float64)).T.astype(np.float32)  # [128, n]
                if kt // 4 == qc:
                    blk[:, 0:128] *= tri
                blks.append(blk)
            wide = np.concatenate(blks, axis=1).astype(FP8E3)  # [128, gn]
            base = off + a * 128 * gn
            out[base:base + 128 * gn] = wide.reshape(-1)
    return out


def _prep_inputs(query, key, value, rel_pos_bias, Wq, Wk, Wv, Wo_w):
    cosT, sinT = _rope_tables()
    xT = {}
    for nm, x in (("q", query), ("k", key)):
        for b in range(B):
            t = np.ascontiguousarray(
                x[b].T.reshape(8, 128, 2, 1024).transpose(1, 2, 0, 3)
            ).astype(BF16)  # [128, half, t, 1024]
            xT[(nm, b)] = t
    for b in range(B):
        t = np.ascontiguousarray(
            value[b].T.reshape(8, 128, 4, 512).transpose(2, 1, 0, 3)
        ).astype(BF16)  # [qc, 128, t, 512]
        xT[("v", b)] = t
    wqs, wks, wvs, wos, ebs = {}, {}, {}, {}, {}
    for g in range(4):
        sl = slice(GDIM * g, GDIM * (g + 1))
        wqs[g] = np.ascontiguousarray(
            (Wq[sl, :] / SCALE).T.reshape(8, 128, GDIM).transpose(1, 0, 2)
        ).astype(BF16)
        wks[g] = np.ascontiguousarray(
            Wk[sl, :].T.reshape(8, 128, GDIM).transpose(1, 0, 2)).astype(BF16)
        wvs[g] = np.ascontiguousarray(
            Wv[sl, :].T.reshape(8, 128, GDIM).transpose(1, 0, 2)).astype(BF16)
        wos[g] = np.ascontiguousarray(
            Wo_w[:, sl].T.reshape(2, 128, D).transpose(1, 0, 2)).astype(BF16)
        ebs[g] = _pack_ebias(rel_pos_bias[0, HPC * g:HPC * (g + 1)])
    in_maps = []
    for c in range(N_CORES):
        b, g = c // 4, c % 4
        in_maps.append({
            "qT": xT[("q", b)], "kT": xT[("k", b)], "vT": xT[("v", b)],
            "wq": wqs[g], "wk": wks[g], "wv": wvs[g], "wo": wos[g],
            "eb": ebs[g], "cosT": cosT, "sinT": sinT,
        })
    return in_maps


def _run(query, key, value, rel_pos_bias, Wq, Wk, Wv, Wo_w, Wo_b, trace=False,
         **trace_kwargs):
    nc = _get_program()
    in_maps = _prep_inputs(query, key, value, rel_pos_bias, Wq, Wk, Wv, Wo_w)
    res = run_bass_kernel_spmd(nc, in_maps, core_ids=list(range(N_CORES)),
                               trace=trace, **trace_kwargs)
    out = np.empty((B, S, D), dtype=np.float32)
    for b in range(B):
        acc = res.results[4 * b]["out"].astype(np.float32)
        for g in range(1, 4):
            acc = acc + res.results[4 * b + g]["out"]
        out[b] = acc + Wo_b[None, :]
    return out, res


def _cpu_fallback(query, key, value, mask, rel_pos_bias, Wq, Wk, Wv, Wo_w, Wo_b):
    def rope_np(x):
        half = DK // 2
        inv_freq = 1.0 / (10000.0 ** (np.arange(half, dtype=np.float32) / half))
        ang = np.arange(S, dtype=np.float32)[:, None] * inv_freq[None, :]
        cos = np.concatenate([np.cos(ang), np.cos(ang)], axis=-1)[None, None]
        sin = np.concatenate([np.sin(ang), np.sin(ang)], axis=-1)[None, None]
        x1, x2 = x[..., :half], x[..., half:]
        rot = np.concatenate([-x2, x1], axis=-1)
        return x * cos + rot * sin

    q = np.einsum('bsd,ed->bse', query, Wq).reshape(B, S, H, DK).transpose(0, 2, 1, 3)
    k = np.einsum('bsd,ed->bse', key, Wk).reshape(B, S, H, DK).transpose(0, 2, 1, 3)
    v = np.einsum('bsd,ed->bse', value, Wv).reshape(B, S, H, DK).transpose(0, 2, 1, 3)
    q, k = rope_np(q), rope_np(k)
    sc = np.einsum('bhqd,bhkd->bhqk', q, k) / SCALE + rel_pos_bias
    sc = np.where(mask, sc, -np.inf)
    sc = sc - sc.max(axis=-1, keepdims=True)
    e = np.exp(sc)
    attn = e / e.sum(axis=-1, keepdims=True)
    ctx = np.einsum('bhqk,bhkd->bhqd', attn, v)
    ctx = ctx.transpose(0, 2, 1, 3).reshape(B, S, D)
    return (np.einsum('bsd,ed->bse', ctx, Wo_w) + Wo_b).astype(np.float32)


def kernel(query, key, value, mask, rel_pos_bias, Wq, Wk, Wv, Wo_w, Wo_b):
    query = np.asarray(query, dtype=np.float32)
    key = np.asarray(key, dtype=np.float32)
    value = np.asarray(value, dtype=np.float32)
    mask = np.asarray(mask)
    rel_pos_bias = np.asarray(rel_pos_bias, dtype=np.float32)
    Wq = np.asarray(Wq, dtype=np.float32)
    Wk = np.asarray(Wk, dtype=np.float32)
    Wv = np.asarray(Wv, dtype=np.float32)
    Wo_w = np.asarray(Wo_w, dtype=np.float32)
    Wo_b = np.asarray(Wo_b, dtype=np.float32)

    if not np.array_equal(mask.reshape(S, S),
                          np.tril(np.ones((S, S), dtype=bool))):
        return _cpu_fallback(query, key, value, mask, rel_pos_bias,
                             Wq, Wk, Wv, Wo_w, Wo_b)

    out, _ = _run(query, key, value, rel_pos_bias, Wq, Wk, Wv, Wo_w, Wo_b)
    return out


# revision 17
# speedup vs baseline: 1.0055x; 1.0006x over previous
"""Multi-head attention (B=2, S=2048, D=1024, H=16, causal + rel-pos-bias + RoPE)
on 8 Trainium2 NeuronCores.

Sharding: core c handles batch c//4 and head-group c%4 (4 heads = 256 model dims).
Each core computes its heads' Q/K/V projections (column-sharded weights), RoPE,
causal attention with relative position bias, and a partial output projection
(row-sharded Wo). Host sums the 4 partials per batch and adds Wo_b.
"""

import math

import numpy as np
import ml_dtypes

import concourse.bass as bass
import concourse.mybir as mybir
import concourse.tile as tile
from concourse import bacc
from concourse.bass_utils import run_bass_kernel_spmd

BF16 = ml_dtypes.bfloat16
FP8E3 = ml_dtypes.float8_e3m4

B, S, D, H = 2, 2048, 1024, 16
DK = 64
SCALE = math.sqrt(DK)
HPC = 4          # heads per core
GDIM = HPC * DK  # 256 model dims per core
N_CORES = 8
KT = S // 128    # 16 k-tiles
QC = S // 512    # 4 q-chunks

f32 = mybir.dt.float32
f32r = mybir.dt.float32r
bf16 = mybir.dt.bfloat16


def _sched():
    """Attention tile schedule, shared by host bias packer and device builder.

    Yields (h, qc, kt, n, q0): head-local index, q-chunk, k-tile, the valid
    column count and starting q of the S^T tile [128 k, n q]."""
    for h in range(HPC):
        for qc in range(QC):
            for kt in range(4 * qc + 4):
                if kt // 4 == qc:  # diagonal-crossing tile
                    n = 512 - 128 * (kt % 4)
                    q0 = 128 * kt
                else:
                    n = 512
                    q0 = 512 * qc
                yield h, qc, kt, n, q0


EB_PER_HEAD = sum(128 * n for h, qc, kt, n, q0 in _sched()) // HPC
EB_TOTAL = EB_PER_HEAD * HPC

_PROGRAM = None


def _quads(qc):
    """kt quad-groups for one (h, qc) chunk: list of [(kt,n,q0)...]."""
    kts = list(range(4 * qc + 4))
    out = []
    for i in range(0, len(kts), 4):
        grp = []
        for kt in kts[i:i + 4]:
            if kt // 4 == qc:
                n = 512 - 128 * (kt % 4)
                q0 = 128 * kt
            else:
                n = 512
                q0 = 512 * qc
            grp.append((kt, n, q0))
        out.append(grp)
    return out


def _build_program():
    nc = bacc.Bacc("TRN2", target_bir_lowering=False, debug=False)

    dqT = nc.dram_tensor("qT", [128, 2, 8, 1024], bf16,
                         kind="ExternalInput").ap()
    dkT = nc.dram_tensor("kT", [128, 2, 8, 1024], bf16,
                         kind="ExternalInput").ap()
    dvT = nc.dram_tensor("vT", [4, 128, 8, 512], bf16,
                         kind="ExternalInput").ap()
    dwq = nc.dram_tensor("wq", [128, 8, GDIM], bf16,
                         kind="ExternalInput").ap()
    dwk = nc.dram_tensor("wk", [128, 8, GDIM], bf16,
                         kind="ExternalInput").ap()
    dwv = nc.dram_tensor("wv", [128, 8, GDIM], bf16,
                         kind="ExternalInput").ap()
    dwo = nc.dram_tensor("wo", [128, 2, D], bf16, kind="ExternalInput").ap()
    deb = nc.dram_tensor("eb", [EB_TOTAL], mybir.dt.float8e3,
                         kind="ExternalInput").ap()
    dcos = nc.dram_tensor("cosT", [128, S], bf16, kind="ExternalInput").ap()
    dsin = nc.dram_tensor("sinT", [128, S], bf16, kind="ExternalInput").ap()
    dout = nc.dram_tensor("out", [S, D], f32, kind="ExternalOutput").ap()

    with tile.TileContext(nc) as tc:
        with tc.tile_pool(name="consts", bufs=1) as consts, \
             tc.tile_pool(name="persist", bufs=1) as persist, \
             tc.tile_pool(name="ropep", bufs=2) as ropep, \
             tc.tile_pool(name="attn_sb", bufs=2) as attn_sb, \
             tc.tile_pool(name="normp", bufs=2) as normp, \
             tc.tile_pool(name="outst", bufs=2) as outst, \
             tc.tile_pool(name="xf", bufs=1) as xf, \
             tc.tile_pool(name="psum", bufs=1, space="PSUM") as psum:

            # ---- constants & resident activations ----
            wq_s = consts.tile([128, 8, GDIM], bf16)
            wk_s = consts.tile([128, 8, GDIM], bf16)
            wv_s = consts.tile([128, 8, GDIM], bf16)
            wo_s = consts.tile([128, 2, D], bf16)
            cos_s = consts.tile([128, S], bf16)
            sin_s = consts.tile([128, S], bf16)
            heat = consts.tile([128, 128], bf16)

            # x quadrants as separate tiles: the 4 DMAs per tensor would
            # otherwise serialize on whole-tile write dependencies
            xp = {}
            for which in ("q", "k"):
                for tq in range(2):
                    for h in range(2):
                        xp[(which, tq, h)] = xf.tile(
                            [128, 4, 1024], bf16, tag=f"x{which}{tq}{h}",
                            name=f"x{which}{tq}{h}")

            # ---- prologue DMA schedule ----
            # All queues share the same 16 SDMA engines (~358 GB/s total), so
            # what matters is DMA SIZE (1MB ~ 78% efficiency vs 32% at 64KB).
            # x moves in ~1MB 4-tile chunks, first halves (which feed the w=0
            # projection waves and qc0/qc1 attention) first.
            nc.scalar.dma_start(out=cos_s, in_=dcos)
            nc.scalar.dma_start(out=sin_s, in_=dsin)
            nc.sync.dma_start(out=wq_s, in_=dwq)
            nc.gpsimd.dma_start(out=wk_s, in_=dwk)
            lo, hi = slice(0, 4), slice(4, 8)
            # x in [p, half, t, 1024] layout: each (tensor, half, t-quad) DMA
            # reads 8KB-contiguous per partition
            for tp in range(2):
                tsl = slice(2 * tp, 2 * tp + 2)
                nc.sync.dma_start(out=xp[("q", 0, 0)][:, tsl, :],
                                  in_=dqT[:, 0, tsl, :])
                nc.gpsimd.dma_start(out=xp[("q", 1, 0)][:, tsl, :],
                                    in_=dqT[:, 0, 2 * tp + 4:2 * tp + 6, :])
            for tp in range(2):
                tsl = slice(2 * tp, 2 * tp + 2)
                nc.sync.dma_start(out=xp[("k", 0, 0)][:, tsl, :],
                                  in_=dkT[:, 0, tsl, :])
                nc.gpsimd.dma_start(out=xp[("k", 1, 0)][:, tsl, :],
                                    in_=dkT[:, 0, 2 * tp + 4:2 * tp + 6, :])
            nc.scalar.dma_start(out=wv_s, in_=dwv)
            nc.gpsimd.dma_start(out=wo_s, in_=dwo)

            # V activations stream in per q-chunk (vf tiles) instead of a
            # resident 4MB buffer: takes them off the prologue critical path.
            def make_vf(qc):
                vft = xf.tile([128, 8, 512], bf16, tag="vf", bufs=2,
                              name=f"vf{qc}")
                eng = nc.scalar if qc < 2 else nc.gpsimd
                eng.dma_start(out=vft, in_=dvT[qc])
                return vft

            vfs = {0: make_vf(0), 1: make_vf(1)}

            # second x halves (feed the deferred w=1 projection waves)
            nc.sync.dma_start(out=xp[("q", 0, 1)], in_=dqT[:, 1, lo, :])
            nc.gpsimd.dma_start(out=xp[("q", 1, 1)], in_=dqT[:, 1, hi, :])
            nc.sync.dma_start(out=xp[("k", 0, 1)], in_=dkT[:, 1, lo, :])
            nc.gpsimd.dma_start(out=xp[("k", 1, 1)], in_=dkT[:, 1, hi, :])

            # PE heater: keep HAM busy during initial DMA wait
            nc.vector.memset(heat, 0.0)
            ones_f = consts.tile([1, DK], f32)
            nc.vector.memset(ones_f, 1.0)
            ones_r = consts.tile([1, DK], f32r)
            nc.vector.tensor_copy(out=ones_r, in_=ones_f)

            QT = [persist.tile([128, S], bf16, name=f"QT{m}") for m in range(2)]
            KTt = [persist.tile([128, S], bf16, name=f"KTt{m}") for m in range(2)]
            Vt = persist.tile([128, KT, HPC, DK + 1], bf16)
            cxT = [persist.tile([128, S], bf16, name=f"cxT{m}") for m in range(2)]
            nc.vector.memset(Vt[:, :, :, DK:DK + 1], 1.0)

            hps = psum.tile([128, 512], f32, tag="sml", bufs=2, name="hps")
            for i in range(20):
                nc.tensor.matmul(hps[:, 0:256], lhsT=heat,
                                 rhs=cos_s[0:128, 0:256],
                                 start=True, stop=True)

            def rope_w(pp, dst, w):
                # pp: 2 psum [128,512] proj.T chunks for cols 1024w:1024w+1024.
                # sin_s rows are host-pre-swapped so each swap-mul reads both
                # inputs at the same base partition (DVE requirement); only
                # the output lands at the swapped 32-block.
                cols = slice(1024 * w, 1024 * w + 1024)
                qb = ropep.tile([128, 1024], bf16, tag="qb", name="qb")
                for j in range(2):
                    nc.scalar.copy(out=qb[:, 512 * j:512 * j + 512], in_=pp[j])
                ss = ropep.tile([128, 1024], bf16, tag="ss", name="ss")
                for base in (0, 64):
                    nc.vector.tensor_mul(out=ss[base:base + 32, :],
                                         in0=qb[base + 32:base + 64, :],
                                         in1=sin_s[base + 32:base + 64, cols])
                    nc.vector.tensor_mul(out=ss[base + 32:base + 64, :],
                                         in0=qb[base:base + 32, :],
                                         in1=sin_s[base:base + 32, cols])
                cc = ropep.tile([128, 1024], bf16, tag="cc", name="cc")
                nc.vector.tensor_mul(out=cc, in0=qb, in1=cos_s[:, cols])
                nc.vector.tensor_add(out=dst[:, cols], in0=cc, in1=ss)

            def proj_block(w):
                # Q/K projections for output cols 1024w:1024w+1024, both m.
                for m in range(2):
                    for which, wsrc, dsts in (("q", wq_s, QT),
                                              ("k", wk_s, KTt)):
                        pp = [psum.tile([128, 512], f32, tag="sml", bufs=2,
                                        name=f"pp{which}{m}{w}{n}")
                              for n in (2 * w, 2 * w + 1)]
                        for t in range(8):
                            xt = xp[(which, t // 4, w)]
                            for j in range(2):
                                nc.tensor.matmul(
                                    pp[j],
                                    lhsT=wsrc[:, t, 128 * m:128 * m + 128],
                                    rhs=xt[:, t % 4, 512 * j:512 * j + 512],
                                    start=(t == 0), stop=(t == 7))
                        rope_w(pp, dsts[m], w)

            def vproj(qc):
                vft = vfs[qc]
                for jj in range(4):
                    tt = 4 * qc + jj
                    pv = psum.tile([128, GDIM], f32, tag="sml", bufs=2,
                                   name="pv")
                    for t in range(8):
                        nc.tensor.matmul(
                            pv,
                            lhsT=vft[:, t, 128 * jj:128 * jj + 128],
                            rhs=wv_s[:, t, :],
                            start=(t == 0), stop=(t == 7))
                    nc.scalar.copy(
                        out=Vt[:, tt, :, 0:DK],
                        in_=pv.rearrange("p (h d) -> p h d", h=HPC))


            def load_eb(m, qc, gi):
                # fp8(e3m4)-packed bias, cast to bf16 in-flight by the SWDGE
                gn, off = EB_OFF[(m, qc, gi)]
                ebt2 = attn_sb.tile([128, 2, 2048], bf16, tag="ebt",
                                    bufs=2, name="ebt2")
                nc.gpsimd.dma_start(
                    out=ebt2[:, :, 0:gn],
                    in_=deb[off:off + 2 * 128 * gn].rearrange(
                        "(a p n) -> p a n", a=2, p=128))
                return ebt2

            # prefetch qc0's bias (one grp per m) ahead of the x second halves
            eb_pre = {m: load_eb(m, 0, 0) for m in range(2)}

            def attn_qc(qc):
                for m in range(2):          # head pair (2m, 2m+1)
                    pcx = [psum.tile([DK + 1, 512], f32, tag="pcx", bufs=2,
                                     name=f"pcx{a}") for a in range(2)]
                    last_kt = 4 * qc + 3
                    for gi, grp in enumerate(_quads(qc)):
                        gn = sum(n for kt, n, q0 in grp)
                        praw = [attn_sb.tile([128, 2048], bf16, tag=f"praw{a}",
                                             bufs=2, name=f"praw{a}")
                                for a in range(2)]
                        if qc == 0 and gi == 0:
                            ebt2 = eb_pre[m]
                        else:
                            ebt2 = load_eb(m, qc, gi)
                        goff = 0
                        for pi in range(0, 4, 2):
                            pair = grp[pi:pi + 2]
                            pss = [psum.tile([128, 1024], f32, tag="pss",
                                             bufs=2, name=f"pss{a}")
                                   for a in range(2)]
                            soff = 0
                            for kt, n, q0 in pair:
                                # a-inner: the two row-tiled matmuls run
                                # concurrently, hiding each other's LDWEIGHTS
                                for a in range(2):
                                    nc.tensor.matmul(
                                        pss[a][:, soff:soff + n],
                                        lhsT=KTt[m][64 * a:64 * a + DK,
                                                    128 * kt:128 * kt + 128],
                                        rhs=QT[m][64 * a:64 * a + DK,
                                                  q0:q0 + n],
                                        start=True, stop=True,
                                        tile_position=(64 * a, 0))
                                soff += n
                            for a in range(2):
                                nc.scalar.activation(
                                    out=praw[a][:, goff:goff + soff],
                                    in_=pss[a][:, 0:soff],
                                    func=mybir.ActivationFunctionType.Exp)
                            goff += soff
                        for a in range(2):
                            nc.vector.tensor_mul(out=praw[a][:, 0:gn],
                                                 in0=praw[a][:, 0:gn],
                                                 in1=ebt2[:, a, 0:gn])
                        goff = 0
                        for kt, n, q0 in grp:
                            co = q0 - 512 * qc
                            for a in range(2):
                                nc.tensor.matmul(
                                    pcx[a][:, co:co + n],
                                    lhsT=Vt[:, kt, 2 * m + a, :],
                                    rhs=praw[a][:, goff:goff + n],
                                    start=(kt == 0), stop=(kt == last_kt))
                            goff += n
                    # normalize: per-head broadcast of the denominator row via
                    # a K=1 matmul, reciprocal on the broadcast, scale ctx.
                    for a in range(2):
                        lone = normp.tile([1, 512], f32r, tag="lone",
                                          name="lone")
                        nc.vector.tensor_copy(out=lone,
                                              in_=pcx[a][DK:DK + 1, :])
                        pb = psum.tile([128, 512], f32, tag="sml", bufs=2,
                                       name="pb")
                        nc.tensor.matmul(pb[0:DK, :], lhsT=ones_r, rhs=lone,
                                         start=True, stop=True)
                        rb = normp.tile([DK, 512], f32, tag="rb", name="rb")
                        nc.vector.reciprocal_approx_fast(out=rb,
                                                         in_=pb[0:DK, :])
                        nc.vector.tensor_mul(
                            out=cxT[m][64 * a:64 * a + DK,
                                       512 * qc:512 * qc + 512],
                            in0=pcx[a][0:DK, :],
                            in1=rb)

                # output projection for this qc's 4 token tiles
                for tt in range(4 * qc, 4 * qc + 4):
                    po = [psum.tile([128, 512], f32, tag="sml", bufs=2,
                                    name=f"po{e}") for e in range(2)]
                    for m in range(2):
                        for e in range(2):
                            nc.tensor.matmul(
                                po[e],
                                lhsT=cxT[m][:, 128 * tt:128 * tt + 128],
                                rhs=wo_s[:, m, 512 * e:512 * e + 512],
                                start=(m == 0), stop=(m == 1))
                    ost = outst.tile([128, D], f32, tag="ost")
                    nc.scalar.copy(out=ost[:, 0:512], in_=po[0])
                    nc.sync.dma_start(
                        out=dout[128 * tt:128 * tt + 128, 0:512],
                        in_=ost[:, 0:512])
                    nc.vector.tensor_copy(out=ost[:, 512:1024], in_=po[1])
                    nc.sync.dma_start(
                        out=dout[128 * tt:128 * tt + 128, 512:1024],
                        in_=ost[:, 512:1024])

            # ---- emission: w0 projections -> qc0 -> w1 projections -> qc1.. ----
            proj_block(0)
            vproj(0)
            attn_qc(0)
            proj_block(1)
            vfs[2] = make_vf(2)
            vproj(1)
            attn_qc(1)
            vfs[3] = make_vf(3)
            vproj(2)
            attn_qc(2)
            vproj(3)
            attn_qc(3)

    nc.compile()
    return nc


def _get_program():
    global _PROGRAM
    if _PROGRAM is None:
        _PROGRAM = _build_program()
    return _PROGRAM


def _rope_tables():
    half = DK // 2
    inv_freq = 1.0 / (10000.0 ** (np.arange(half, dtype=np.float64) / half))
    ang = np.arange(S, dtype=np.float64)[:, None] * inv_freq[None, :]  # [S, 32]
    cos = np.cos(ang).T  # [32, S]
    sin = np.sin(ang).T
    cos64 = np.concatenate([cos, cos], axis=0)            # [64, S]
    # signed for rotate-half AND half-swapped: the device reads the sin row at
    # the SOURCE partition of each swap-mul (row p holds the coefficient that
    # multiplies data row p before it lands at the swapped position).
    sin64 = np.concatenate([sin, -sin], axis=0)
    cosT = np.tile(cos64, (2, 1)).astype(BF16)            # [128, S]
    sinT = np.tile(sin64, (2, 1)).astype(BF16)
    return np.ascontiguousarray(cosT), np.ascontiguousarray(sinT)


def _eb_layout():
    """Yields (m, qc, gi, grp, gn, off): the packed-bias block layout. Each
    block holds BOTH heads of the m-pair ([a, 128, gn] contiguous) so the
    device loads it with a single ~1MB DMA."""
    off = 0
    for m in range(2):
        for qc in range(QC):
            for gi, grp in enumerate(_quads(qc)):
                gn = sum(n for kt, n, q0 in grp)
                yield m, qc, gi, grp, gn, off
                off += 2 * 128 * gn


EB_OFF = {(m, qc, gi): (gn, off)
          for m, qc, gi, grp, gn, off in _eb_layout()}


def _pack_ebias(bias_g):
    """bias_g: [HPC, S, S] f32 (this group's heads). Returns packed 1D fp8
    (e3m4), one contiguous [a, 128, gn] block per (m, qc, kt-quad)."""
    out = np.empty(EB_TOTAL, dtype=FP8E3)
    tri = np.triu(np.ones((128, 128), dtype=np.float32))
    for m, qc, gi, grp, gn, off in _eb_layout():
        for a in range(2):
            h = 2 * m + a
            blks = []
            for kt, n, q0 in grp:
                blk = np.exp(
                    bias_g[h, q0:q0 + n, 128 * kt:128 * kt + 128]
                    .astype(np.float64)).T.astype(np.float32)  # [128, n]
                if kt // 4 == qc:
                    blk[:, 0:128] *= tri
                blks.append(blk)
            wide = np.concatenate(blks, axis=1).astype(FP8E3)  # [128, gn]
            base = off + a * 128 * gn
            out[base:base + 128 * gn] = wide.reshape(-1)
    return out


def _prep_inputs(query, key, value, rel_pos_bias, Wq, Wk, Wv, Wo_w):
    cosT, sinT = _rope_tables()
    xT = {}
    for nm, x in (("q", query), ("k", key)):
        for b in range(B):
            t = np.ascontiguousarray(
                x[b].T.reshape(8, 128, 2, 1024).transpose(1, 2, 0, 3)
            ).astype(BF16)  # [128, half, t, 1024]
            xT[(nm, b)] = t
    for b in range(B):
        t = np.ascontiguousarray(
            value[b].T.reshape(8, 128, 4, 512).transpose(2, 1, 0, 3)
        ).astype(BF16)  # [qc, 128, t, 512]
        xT[("v", b)] = t
    wqs, wks, wvs, wos, ebs = {}, {}, {}, {}, {}
    for g in range(4):
        sl = slice(GDIM * g, GDIM * (g + 1))
        wqs[g] = np.ascontiguousarray(
            (Wq[sl, :] / SCALE).T.reshape(8, 128, GDIM).transpose(1, 0, 2)
        ).astype(BF16)
        wks[g] = np.ascontiguousarray(
            Wk[sl, :].T.reshape(8, 128, GDIM).transpose(1, 0, 2)).astype(BF16)
        wvs[g] = np.ascontiguousarray(
            Wv[sl, :].T.reshape(8, 128, GDIM).transpose(1, 0, 2)).astype(BF16)
        wos[g] = np.ascontiguousarray(
            Wo_w[:, sl].T.reshape(2, 128, D).transpose(1, 0, 2)).astype(BF16)
        ebs[g] = _pack_ebias(rel_pos_bias[0, HPC * g:HPC * (g + 1)])
    in_maps = []
    for c in range(N_CORES):
        b, g = c // 4, c % 4
        in_maps.append({
            "qT": xT[("q", b)], "kT": xT[("k", b)], "vT": xT[("v", b)],
            "wq": wqs[g], "wk": wks[g], "wv": wvs[g], "wo": wos[g],
            "eb": ebs[g], "cosT": cosT, "sinT": sinT,
        })
    return in_maps


def _run(query, key, value, rel_pos_bias, Wq, Wk, Wv, Wo_w, Wo_b, trace=False,
         **trace_kwargs):
    nc = _get_program()
    in_maps = _prep_inputs(query, key, value, rel_pos_bias, Wq, Wk, Wv, Wo_w)
    res = run_bass_kernel_spmd(nc, in_maps, core_ids=list(range(N_CORES)),
                               trace=trace, **trace_kwargs)
    out = np.empty((B, S, D), dtype=np.float32)
    for b in range(B):
        acc = res.results[4 * b]["out"].astype(np.float32)
        for g in range(1, 4):
            acc = acc + res.results[4 * b + g]["out"]
        out[b] = acc + Wo_b[None, :]
    return out, res


def _cpu_fallback(query, key, value, mask, rel_pos_bias, Wq, Wk, Wv, Wo_w, Wo_b):
    def rope_np(x):
        half = DK // 2
        inv_freq = 1.0 / (10000.0 ** (np.arange(half, dtype=np.float32) / half))
        ang = np.arange(S, dtype=np.float32)[:, None] * inv_freq[None, :]
        cos = np.concatenate([np.cos(ang), np.cos(ang)], axis=-1)[None, None]
        sin = np.concatenate([np.sin(ang), np.sin(ang)], axis=-1)[None, None]
        x1, x2 = x[..., :half], x[..., half:]
        rot = np.concatenate([-x2, x1], axis=-1)
        return x * cos + rot * sin

    q = np.einsum('bsd,ed->bse', query, Wq).reshape(B, S, H, DK).transpose(0, 2, 1, 3)
    k = np.einsum('bsd,ed->bse', key, Wk).reshape(B, S, H, DK).transpose(0, 2, 1, 3)
    v = np.einsum('bsd,ed->bse', value, Wv).reshape(B, S, H, DK).transpose(0, 2, 1, 3)
    q, k = rope_np(q), rope_np(k)
    sc = np.einsum('bhqd,bhkd->bhqk', q, k) / SCALE + rel_pos_bias
    sc = np.where(mask, sc, -np.inf)
    sc = sc - sc.max(axis=-1, keepdims=True)
    e = np.exp(sc)
    attn = e / e.sum(axis=-1, keepdims=True)
    ctx = np.einsum('bhqk,bhkd->bhqd', attn, v)
    ctx = ctx.transpose(0, 2, 1, 3).reshape(B, S, D)
    return (np.einsum('bsd,ed->bse', ctx, Wo_w) + Wo_b).astype(np.float32)


def kernel(query, key, value, mask, rel_pos_bias, Wq, Wk, Wv, Wo_w, Wo_b):
    query = np.asarray(query, dtype=np.float32)
    key = np.asarray(key, dtype=np.float32)
    value = np.asarray(value, dtype=np.float32)
    mask = np.asarray(mask)
    rel_pos_bias = np.asarray(rel_pos_bias, dtype=np.float32)
    Wq = np.asarray(Wq, dtype=np.float32)
    Wk = np.asarray(Wk, dtype=np.float32)
    Wv = np.asarray(Wv, dtype=np.float32)
    Wo_w = np.asarray(Wo_w, dtype=np.float32)
    Wo_b = np.asarray(Wo_b, dtype=np.float32)

    if not np.array_equal(mask.reshape(S, S),
                          np.tril(np.ones((S, S), dtype=bool))):
        return _cpu_fallback(query, key, value, mask, rel_pos_bias,
                             Wq, Wk, Wv, Wo_w, Wo_b)

    out, _ = _run(query, key, value, rel_pos_bias, Wq, Wk, Wv, Wo_w, Wo_b)
    return out
